# revision 36
# baseline (speedup 1.0000x reference)
"""Causal self-attention (B=2, T=2048, C=1024, NH=16, HD=64) on 8 TRN2 cores.

Sharding: core c -> batch b = c//4, head group j = c%4 (4 heads: 4j..4j+3).
Each core computes its batch's QKV projection for its 4 heads, rope, causal
flash-style attention in S^T layout (k on partitions, q on free dim), and a
partial output projection y_part^T = Wp_slice^T.T @ attT. Host sums the 4
per-batch partials (bf16) in f32 and adds b_proj.

Schedule (single pass, PE kept saturated):
  - x is loaded in 512-column chunks (t4-major layout) so the first QK block
    starts after ~2.3MB of DMA instead of the full 8MB.
  - Phase D (attention) per head pair; per 512-col q-chunk qc the rowsum
    lands in the PV psum (ones column), and normalization runs incrementally:
    reciprocal -> PE broadcast (contraction-2 matmul with a 0/1 ones2 matrix)
    -> attbf = attT * rsb. No DRAM bounce, no end-of-phase stall.
  - Phase E (output proj) is emitted per 512-col chunk as fillers inside D1
    as soon as that chunk's hp1 normalize is done; yT streams out in bf16.

Device layouts (per core, t = 2048 tokens of its batch):
  xT   [128, 4, 8, 512] bf16  x[b].T tiled: [c-part, t4-chunk, c-tile, 512]
  q/k  [128, 2, 2048]  bf16   head-pair dims on partitions, rope applied
  v    [128, 64, 65]   bf16   [tok-part, u=(tt,hp,h), 64 dims + ones col]
  S^T  psum [128, 1024] f32   per kt: [h0 512 | h1 512]
  P^T  [128, 1024] bf16       exp(S^T/8), causal-masked (mask mul on DVE)
  PV   psum [65, 512]         rows 0-63 att^T, row 64 rowsum (ones col)
  attT [128, 2, 2048] f32 -> attbf = attT * bcast(1/rowsum) bf16
  yT   [1024, 2048] bf16      y_part^T = WpT.T @ attbf, streamed per chunk
"""
import numpy as np
import ml_dtypes
from contextlib import ExitStack

import concourse.bass as bass
import concourse.mybir as mybir
import concourse.tile as tile
from concourse import bacc
from concourse.bass_utils import run_bass_kernel_spmd

F32 = mybir.dt.float32
F16 = mybir.dt.float16
BF16 = mybir.dt.bfloat16
AF = mybir.ActivationFunctionType
ALU = mybir.AluOpType

B, T, C = 2, 2048, 1024
NH, HD = 16, 64
TL = 2048          # per-core token count (one batch)
NCT = C // 128     # 8 contraction tiles
NTC = TL // 512    # 4 t-chunks of 512
NTT = TL // 128    # 16 token tiles of 128

TRACE = False      # set by test harness for profiling runs
_CACHE = {}


def _build_nc():
    nc = bacc.Bacc("TRN2", target_bir_lowering=False, debug=False)
    xT_d = nc.dram_tensor("xT", [128, NTC, NCT, 512], BF16, kind="ExternalInput").ap()
    wqk_d = nc.dram_tensor("wqkT", [128, 4, NCT, 128], BF16, kind="ExternalInput").ap()
    wv_d = nc.dram_tensor("wvT", [128, NCT, 256], BF16, kind="ExternalInput").ap()
    bqk_d = nc.dram_tensor("bqk", [128, 4], F32, kind="ExternalInput").ap()
    bv_d = nc.dram_tensor("bv", [128, 256], F32, kind="ExternalInput").ap()
    rope_d = nc.dram_tensor("rope", [128, NTC, 512], BF16, kind="ExternalInput").ap()
    wp_d = nc.dram_tensor("wpT", [128, 2, 1024], BF16, kind="ExternalInput").ap()
    ones1_d = nc.dram_tensor("ones1", [33, 64], BF16, kind="ExternalInput").ap()
    ones1f_d = nc.dram_tensor("ones1f", [33, 64], F32, kind="ExternalInput").ap()
    yT_d = nc.dram_tensor("yT", [1024, TL], BF16, kind="ExternalOutput").ap()

    with tile.TileContext(nc) as tc, ExitStack() as ctx:
        sb = ctx.enter_context(tc.tile_pool(name="sb", bufs=1))
        ptp = ctx.enter_context(tc.tile_pool(name="ptp", bufs=6))
        ytp = ctx.enter_context(tc.tile_pool(name="ytp", bufs=8))

        xT = sb.tile([128, NTC, NCT, 512], BF16)
        wqk = sb.tile([128, 4, NCT, 128], BF16)
        wv = sb.tile([128, NCT, 256], BF16)
        bqk = sb.tile([128, 4], F32)
        bv = sb.tile([128, 256], F32)
        rope = sb.tile([128, NTC, 512], BF16)
        mi = sb.tile([128, 4, 1024], F16)  # c - p - 128d; mask = (mi >= 0)
        wp = sb.tile([128, 2, 1024], BF16)
        q_sb = sb.tile([128, 2, TL], BF16)
        k_sb = sb.tile([128, 2, TL], BF16)
        v_sb = sb.tile([128, 4 * NTT, 65], BF16)
        attT = sb.tile([128, 2, TL], F32)
        attbf = sb.tile([128, 2, TL], BF16)
        rs2 = sb.tile([128, TL], F32)    # rows 32*h hold chunk rowsums
        rs2r = sb.tile([128, TL], F32)
        ones1 = sb.tile([33, 64], BF16)  # bcast lhsT: ones at rows 0 and 32
        ones1f = sb.tile([33, 64], F32)  # f32 twin for the final (tail) chunk
        rs2rb = sb.tile([33, TL], BF16)  # bf16 reciprocal rows for the bcast

        # critical path first: B block0 needs wqk m=2,0 + x t4=0 + rope0 +
        # bqk; C tiles 0-3 need wv/bv. Later tensors stream behind on the 4
        # DGE rings in need order. Masks are computed on the (idle) gpsimd
        # engine instead of DMA: val[p,c] = c - p - 128d, mask = (val >= 0).
        nc.sync.dma_start(out=wqk[:, 2], in_=wqk_d[:, 2])
        nc.gpsimd.dma_start(out=wqk[:, 0], in_=wqk_d[:, 0])
        nc.scalar.dma_start(out=bqk, in_=bqk_d)
        nc.scalar.dma_start(out=rope[:, 0], in_=rope_d[:, 0])
        nc.sync.dma_start(out=xT[:, 0, 0:4], in_=xT_d[:, 0, 0:4])
        nc.gpsimd.dma_start(out=xT[:, 0, 4:8], in_=xT_d[:, 0, 4:8])
        nc.sync.dma_start(out=wqk[:, 3], in_=wqk_d[:, 3])
        nc.scalar.dma_start(out=wv, in_=wv_d)
        nc.scalar.dma_start(out=bv, in_=bv_d)
        nc.vector.memset(v_sb[:, :, 64:65], 1.0)
        nc.vector.memset(rs2, 1.0)  # keep unused partitions finite for recip
        for d in range(4):
            nc.gpsimd.iota(mi[:, d, :], [[0, 2], [1, 512]], base=-128 * d,
                           channel_multiplier=-1,
                           allow_small_or_imprecise_dtypes=True)
            if d == 0:
                nc.gpsimd.dma_start(out=wqk[:, 1], in_=wqk_d[:, 1])
            elif d == 1:
                nc.sync.dma_start(out=xT[:, 1], in_=xT_d[:, 1])
            elif d == 2:
                nc.gpsimd.dma_start(out=xT[:, 3], in_=xT_d[:, 3])
        nc.sync.dma_start(out=xT[:, 2], in_=xT_d[:, 2])
        for t4 in range(1, NTC):
            nc.scalar.dma_start(out=rope[:, t4], in_=rope_d[:, t4])
        nc.scalar.dma_start(out=ones1, in_=ones1_d)
        nc.scalar.dma_start(out=ones1f, in_=ones1f_d)
        nc.scalar.dma_start(out=wp, in_=wp_d)

        def phase_b_block(ms, t4, pool):
            """QK projection for one 512-col t-chunk of the given m-tiles."""
            sl = slice(t4 * 512, (t4 + 1) * 512)
            for fi, m in enumerate(ms):
                ps = pool.tile([128, 512], F32, tag=f"f{fi}", name=f"pbb_{m}_{t4}")
                for ct in range(NCT):
                    nc.tensor.matmul(
                        ps, wqk[:, m, ct, :], xT[:, t4, ct, :],
                        start=(ct == 0), stop=(ct == NCT - 1),
                        skip_group_check=True)
                dest = q_sb if m < 2 else k_sb
                nc.vector.scalar_tensor_tensor(
                    out=dest[:, m % 2, sl], in0=ps, scalar=bqk[:, m:m + 1],
                    in1=rope[:, t4, :], op0=ALU.add, op1=ALU.mult)

        def phase_b_half(m, t4, half, state, pb, tag):
            """One half (4 of 8 contraction MMs) of the QK projection for
            (m-tile, t-chunk); the second half adds the bias*rope STT."""
            if half == 0:
                state[(m, t4)] = pb.tile(
                    [128, 512], F32, tag=tag, name=f"pbf_{m}_{t4}")
            ps = state[(m, t4)]
            for ct in range(4 * half, 4 * half + 4):
                nc.tensor.matmul(
                    ps, wqk[:, m, ct, :], xT[:, t4, ct, :],
                    start=(ct == 0), stop=(ct == NCT - 1),
                    skip_group_check=True)
            if half == 1:
                dest = q_sb if m < 2 else k_sb
                sl = slice(t4 * 512, (t4 + 1) * 512)
                nc.vector.scalar_tensor_tensor(
                    out=dest[:, m % 2, sl], in0=ps, scalar=bqk[:, m:m + 1],
                    in1=rope[:, t4, :], op0=ALU.add, op1=ALU.mult)

        def phase_c_tile(tt, pool):
            """V projection for one token tile (8 MMs + fused bias add)."""
            ps = pool.tile([128, 512], F32, tag=f"f{tt % 2}", name=f"pc_{tt}")[:, 0:256]
            t4, r = divmod(tt, 4)
            for ct in range(NCT):
                nc.tensor.matmul(
                    ps, xT[:, t4, ct, r * 128:(r + 1) * 128], wv[:, ct, :],
                    start=(ct == 0), stop=(ct == NCT - 1),
                    skip_group_check=True)
            # one fused add for all 4 units of this token tile:
            # psum cols (hp*128+h*64) map to v_sb units u=4tt+2hp+h in order
            nc.vector.tensor_add(
                v_sb[:, 4 * tt:4 * tt + 4, 0:64],
                ps.rearrange("p (a b) -> p a b", a=4),
                bv.rearrange("p (a b) -> p a b", a=4))

        def norm_chunk(hp, qc, pool, last=False):
            """attbf[:, hp, qc-chunk] = attT * 1/rowsum. The reciprocal rows
            (head h at partition 32*h -- engine APs only allow base
            partitions 0/32/64; rows are reused across the temporally
            separated head pairs) are broadcast across the two 64-partition
            head blocks by contraction-1 PE matmuls with a ones row -- no
            DRAM bounce.
            reciprocal_approx_fast runs full-partition (custom-DVE op
            mishandles partition-sliced APs); other rows are finite (memset).
            """
            qsl = slice(qc * 512, (qc + 1) * 512)
            nc.vector.reciprocal_approx_fast(rs2r[:, qsl], rs2[:, qsl])
            rsb = pool.tile([128, 512], F32, tag="f0", name=f"rsb_{hp}_{qc}")
            for h in range(2):
                r = 32 * h
                if last:  # skip the converts on the drain-critical chain
                    nc.tensor.matmul(rsb[h * 64:(h + 1) * 64, :],
                                     ones1f[r:r + 1, :], rs2r[r:r + 1, qsl],
                                     skip_group_check=True)
                    continue
                # bf16 moving operand: 1 PE cycle/row instead of f32's 4
                nc.vector.tensor_copy(rs2rb[r:r + 1, qsl], rs2r[r:r + 1, qsl])
                nc.tensor.matmul(rsb[h * 64:(h + 1) * 64, :],
                                 ones1[r:r + 1, :], rs2rb[r:r + 1, qsl],
                                 skip_group_check=True)
            nc.vector.tensor_mul(attbf[:, hp, qsl], attT[:, hp, qsl], rsb)

        def phase_d(hp, pds, pdv, fill, fillers, copies_fn,
                    pre_qc=None, qc_order=(0, 1, 2, 3)):
            """Attention for head pair hp. One kt per S^T/exp group so the
            multi-buffered psum keeps PE(S) / ACT(exp) / PE(PV) overlapped.
            Diagonal tiles d = kt-4qc only have live entries in q-columns
            >= 128d, so S/exp/mask/PV are all narrowed to [128d:512) per
            head (exp/mask via a 3D two-block AP). fillers: independent PE
            work chunks consumed greedily (one per kt, destructively so the
            caller sees leftovers). Normalization runs per qc right after
            the PV copies. att_copy: engine callable for the psum->attT
            drain (scalar in D0 where exp has slack, vector in D1)."""
            for qi, qc in enumerate(qc_order):
                if pre_qc is not None:
                    pre_qc(qi)
                qsl = slice(qc * 512, (qc + 1) * 512)
                pv = [pdv.tile([65, 512], F32, tag=f"pv{h}", name=f"pv_{qc}_{hp}_{h}")
                      for h in range(2)]
                n_kt = 4 * (qc + 1)
                for _ in range(2 if n_kt <= 4 else 1):
                    if fillers:
                        fillers.pop(0)()  # ahead of S(kt0): covers psum waits
                qstride = max(1, n_kt // len(fillers)) if fillers else 1
                for kt in range(n_kt):
                    ksl = slice(kt * 128, (kt + 1) * 128)
                    d = kt - 4 * qc if kt >= 4 * qc else None
                    off = 128 * d if d else 0
                    sps = pds.tile([128, 1024], F32, tag="sps", name=f"sps_{qc}_{hp}_{kt}")
                    for h in range(2):
                        hsl = slice(h * 64, (h + 1) * 64)
                        nc.tensor.matmul(
                            sps[:, h * 512 + off:(h + 1) * 512],
                            k_sb[hsl, hp, ksl],
                            q_sb[hsl, hp, qc * 512 + off:(qc + 1) * 512])
                    pt = ptp.tile([128, 1024], BF16, tag="pt", name=f"pt_{qc}_{hp}_{kt}")
                    if off:
                        pt_r = pt.rearrange("p (h c) -> p h c", h=2)[:, :, off:512]
                        sps_r = sps.rearrange("p (h c) -> p h c", h=2)[:, :, off:512]
                        nc.scalar.activation(pt_r, sps_r, AF.Exp, bias=0.0, scale=0.125)
                    else:
                        nc.scalar.activation(pt, sps, AF.Exp, bias=0.0, scale=0.125)
                    if d is not None:  # partial (diagonal) tile: mask both heads
                        pt_m = pt.rearrange("p (h c) -> p h c", h=2)[:, :, off:512]
                        msk = mi[:, d, :].rearrange("p (h c) -> p h c", h=2)[:, :, off:512]
                        nc.vector.scalar_tensor_tensor(
                            out=pt_m, in0=msk, scalar=0.0, in1=pt_m,
                            op0=ALU.is_ge, op1=ALU.mult)
                    for h in range(2):
                        u = (kt * 2 + hp) * 2 + h
                        nc.tensor.matmul(
                            pv[h][:, off:512], v_sb[:, u, :],
                            pt[:, h * 512 + off:(h + 1) * 512],
                            start=(kt == 0), stop=(kt == n_kt - 1),
                            skip_group_check=True)
                    if fillers and kt % qstride == qstride - 1:
                        fillers.pop(0)()
                att_copy, rs_copy = copies_fn(qi)
                for h in range(2):
                    r = 32 * h
                    rs_copy(rs2[r:r + 1, qsl], pv[h][64:65, :])
                for h in range(2):
                    att_copy(attT[h * 64:(h + 1) * 64, hp, qsl], pv[h][0:64, :])
                norm_chunk(hp, qc, fill, last=(hp == 1 and qi == 3))

        def phase_e_pair(mp, t4, pool, copy_engines, tail=False):
            """Output projection for 2 m-tiles of one 512-col t-chunk:
            y^T = WpT.T @ attbf, psum -> bf16 -> DRAM."""
            for i in range(2):
                mt = 2 * mp + i
                ps = pool.tile([128, 512], F32, tag=f"f{(mp + i) % 2}",
                               name=f"pe_{t4}_{mt}")
                for hp in range(2):
                    nc.tensor.matmul(
                        ps, wp[:, hp, mt * 128:(mt + 1) * 128],
                        attbf[:, hp, t4 * 512:(t4 + 1) * 512],
                        start=(hp == 0), stop=(hp == 1), skip_group_check=True)
                yt = ytp.tile([128, 512], BF16, tag="yt", name=f"yt_{mt}_{t4}")
                copy_engines[i](yt, ps)
                ring = (nc.sync, nc.gpsimd)[i]
                ring.dma_start(
                    out=yT_d[mt * 128:(mt + 1) * 128, t4 * 512:(t4 + 1) * 512],
                    in_=yt)

        with tc.tile_pool(name="fill", bufs=1, space="PSUM") as fill, \
             tc.tile_pool(name="pds", bufs=2, space="PSUM") as pds, \
             tc.tile_pool(name="pdv", bufs=1, space="PSUM") as pdv:
            # head-pair-0 QK t-chunk 0 + v tiles for D0 qc0, emitted up front
            phase_b_block((2, 0), 0, fill)
            for tt in range(4):
                phase_c_tile(tt, fill)
            # remaining v tiles + head-pair-1 QK interleave into D0's gaps,
            # ordered by the x t-chunk they need so the filler stream never
            # outpaces the input DMA stream (PE FIFO head-of-line blocking)
            bstate = {}
            fillers = []
            fi = [0]

            def b_half(m, t4, half):
                tag = f"f{fi[0] % 2}"
                if half == 1:
                    fi[0] += 1
                return lambda: phase_b_half(m, t4, half, bstate, fill, tag)

            # D0 fillers: B-hp1 t-chunk 0 (must precede D1 qc0) + remaining v
            # tiles, grouped by the x t-chunk they need so the filler stream
            # never outpaces the input DMA stream.
            for m in (3, 1):
                for half in range(2):
                    fillers.append(b_half(m, 0, half))
            for tt in range(4, NTT):
                fillers.append(lambda tt=tt: phase_c_tile(tt, fill))

            def b0_pre(qc):
                if qc > 0:
                    phase_b_block((2, 0), qc, fill)

            # copies: scalar while its exp pipe has slack; the last qc's
            # copies feed the D0->D1 boundary, keep them off scalar's queue
            phase_d(0, pds, pdv, fill, fillers,
                    lambda qi: ((nc.vector.tensor_copy, nc.vector.tensor_copy)
                                if qi == 3 else (nc.scalar.copy, nc.scalar.copy)),
                    pre_qc=b0_pre)
            for f in list(fillers):
                f()
            fillers.clear()

            # D1 is exp-bound: feed it the remaining B-hp1 chunks (t-chunk k
            # only needed from qc k on) and phase E per chunk as soon as that
            # chunk's normalize has run. E psum->bf16 copies go to vector
            # while exp is hot, scalar once exp drains.
            e_fillers = []
            for m in (3, 1):
                for half in range(2):
                    e_fillers.append(b_half(m, 1, half))

            def d1_pre(qc):
                if qc in (1, 2):
                    for m in (3, 1):
                        for half in range(2):
                            e_fillers.append(b_half(m, qc + 1, half))
                if qc > 0:
                    ce = (nc.vector.tensor_copy, nc.vector.tensor_copy)
                    e_fillers.extend(
                        (lambda mp=mp, t4=qc - 1: phase_e_pair(mp, t4, fill, ce))
                        for mp in range(4))

            # last qc: attT drain on (now idle) scalar in parallel with the
            # vector-side rs->recip->bcast chain that gates the final E chunk
            phase_d(1, pds, pdv, fill, e_fillers,
                    lambda qi: ((nc.scalar.copy, nc.vector.tensor_copy)
                                if qi == 3 else
                                (nc.vector.tensor_copy, nc.vector.tensor_copy)),
                    pre_qc=d1_pre)
            for f in list(e_fillers):
                f()
            e_fillers.clear()
        # final E chunk: pds/pdv banks are free now; a 4-buffer psum pool
        # removes the 2-bank copy/matmul round-robin stall in the drain
        with tc.tile_pool(name="pe4", bufs=2, space="PSUM") as pe4:
            for mp in range(4):
                phase_e_pair(mp, 3, pe4, (nc.vector.tensor_copy, nc.scalar.copy),
                             tail=True)
    nc.compile()
    return nc


def _rope_T():
    theta = 1.0 / (10000.0 ** (2.0 * np.arange(0, HD // 2, dtype=np.float32) / HD))
    seq = np.arange(1, T + 1, dtype=np.float32)
    ang = np.einsum('n,d->nd', seq, theta)
    ang = np.concatenate([ang, ang], axis=-1)
    f = (np.cos(ang) + np.sin(ang)).astype(np.float32)  # [T, 64]
    return np.concatenate([f.T, f.T], axis=0)           # [128, T]


def _host_inputs(x, W_attn, b_attn, W_proj, b_proj):
    bf = ml_dtypes.bfloat16
    ropeT = np.ascontiguousarray(_rope_T().reshape(128, NTC, 512))
    ones1 = np.zeros((33, 64), bf)
    ones1[0, :] = 1.0
    ones1[32, :] = 1.0
    ones1f = ones1.astype(np.float32)

    in_maps = []
    for c in range(8):
        b, j = divmod(c, 4)
        hs = [4 * j + i for i in range(4)]
        xT = np.ascontiguousarray(x[b].T).astype(bf)          # [1024, TL]
        q_rows = np.concatenate([W_attn[64 * h:64 * (h + 1)] for h in hs], 0)
        k_rows = np.concatenate([W_attn[C + 64 * h:C + 64 * (h + 1)] for h in hs], 0)
        WqkT = np.concatenate([q_rows, k_rows], 0).T          # [1024, 512]
        bqk = np.concatenate(
            [np.concatenate([b_attn[64 * h:64 * (h + 1)] for h in hs]),
             np.concatenate([b_attn[C + 64 * h:C + 64 * (h + 1)] for h in hs])])
        v_rows = np.concatenate([W_attn[2 * C + 64 * h:2 * C + 64 * (h + 1)] for h in hs], 0)
        WvT = v_rows.T                                        # [1024, 256]
        bv = np.concatenate([b_attn[2 * C + 64 * h:2 * C + 64 * (h + 1)] for h in hs])
        WpT = np.concatenate([W_proj[:, 64 * h:64 * (h + 1)] for h in hs], 1).T  # [256,1024]
        in_maps.append({
            "xT": np.ascontiguousarray(
                xT.reshape(NCT, 128, NTC, 512).transpose(1, 2, 0, 3)),
            "wqkT": np.ascontiguousarray(
                WqkT.astype(bf).reshape(NCT, 128, 4, 128).transpose(1, 2, 0, 3)),
            "wvT": np.ascontiguousarray(
                WvT.astype(bf).reshape(NCT, 128, 256).transpose(1, 0, 2)),
            "bqk": np.ascontiguousarray(bqk.reshape(4, 128).T.astype(np.float32)),
            "bv": np.ascontiguousarray(
                np.broadcast_to(bv[None, :].astype(np.float32), (128, 256))),
            "rope": ropeT.astype(bf),
            "ones1": ones1,
            "ones1f": ones1f,
            "wpT": np.ascontiguousarray(
                WpT.astype(bf).reshape(2, 128, 1024).transpose(1, 0, 2)),
        })
    return in_maps


def kernel(x, W_attn, b_attn, W_proj, b_proj):
    if "nc" not in _CACHE:
        _CACHE["nc"] = _build_nc()
    nc = _CACHE["nc"]
    in_maps = _host_inputs(x, W_attn, b_attn, W_proj, b_proj)
    res = run_bass_kernel_spmd(nc, in_maps, list(range(8)), trace=TRACE)
    _CACHE["last"] = res
    y = np.zeros((B, T, C), np.float32)
    for c in range(8):
        y[c // 4] += res.results[c]["yT"].astype(np.float32).T
    y += b_proj.astype(np.float32)
    return y


# revision 37
# speedup vs baseline: 1.0187x; 1.0187x over previous
"""Causal self-attention (B=2, T=2048, C=1024, NH=16, HD=64) on 8 TRN2 cores.

Sharding: core c -> batch b = c//4, head group j = c%4 (4 heads: 4j..4j+3).
Each core computes its batch's QKV projection for its 4 heads, rope, causal
flash-style attention in S^T layout (k on partitions, q on free dim), and a
partial output projection y_part^T = Wp_slice^T.T @ attT. Host sums the 4
per-batch partials (bf16) in f32 and adds b_proj.

Schedule (single pass, PE kept saturated):
  - x is loaded in 512-column chunks (t4-major layout) so the first QK block
    starts after ~2.3MB of DMA instead of the full 8MB.
  - Phase D (attention) per head pair; per 512-col q-chunk qc the rowsum
    lands in the PV psum (ones column), and normalization runs incrementally:
    reciprocal -> PE broadcast (contraction-2 matmul with a 0/1 ones2 matrix)
    -> attbf = attT * rsb. No DRAM bounce, no end-of-phase stall.
  - Phase E (output proj) is emitted per 512-col chunk as fillers inside D1
    as soon as that chunk's hp1 normalize is done; yT streams out in bf16.

Device layouts (per core, t = 2048 tokens of its batch):
  xT   [128, 4, 8, 512] bf16  x[b].T tiled: [c-part, t4-chunk, c-tile, 512]
  q/k  [128, 2, 2048]  bf16   head-pair dims on partitions, rope applied
  v    [128, 64, 65]   bf16   [tok-part, u=(tt,hp,h), 64 dims + ones col]
  S^T  psum [128, 1024] f32   per kt: [h0 512 | h1 512]
  P^T  [128, 1024] bf16       exp(S^T/8), causal-masked (mask mul on DVE)
  PV   psum [65, 512]         rows 0-63 att^T, row 64 rowsum (ones col)
  attT [128, 2, 2048] f32 -> attbf = attT * bcast(1/rowsum) bf16
  yT   [1024, 2048] bf16      y_part^T = WpT.T @ attbf, streamed per chunk
"""
import numpy as np
import ml_dtypes
from contextlib import ExitStack

import concourse.bass as bass
import concourse.mybir as mybir
import concourse.tile as tile
from concourse import bacc
from concourse.bass_utils import run_bass_kernel_spmd

F32 = mybir.dt.float32
F16 = mybir.dt.float16
BF16 = mybir.dt.bfloat16
AF = mybir.ActivationFunctionType
ALU = mybir.AluOpType

B, T, C = 2, 2048, 1024
NH, HD = 16, 64
TL = 2048          # per-core token count (one batch)
NCT = C // 128     # 8 contraction tiles
NTC = TL // 512    # 4 t-chunks of 512
NTT = TL // 128    # 16 token tiles of 128

TRACE = False      # set by test harness for profiling runs
_CACHE = {}


def _build_nc():
    nc = bacc.Bacc("TRN2", target_bir_lowering=False, debug=False)
    xT_d = nc.dram_tensor("xT", [128, NTC, NCT, 512], BF16, kind="ExternalInput").ap()
    wqk_d = nc.dram_tensor("wqkT", [128, 4, NCT, 128], BF16, kind="ExternalInput").ap()
    wv_d = nc.dram_tensor("wvT", [128, NCT, 256], BF16, kind="ExternalInput").ap()
    bqk_d = nc.dram_tensor("bqk", [128, 4], F32, kind="ExternalInput").ap()
    bv_d = nc.dram_tensor("bv", [128, 256], F32, kind="ExternalInput").ap()
    rope_d = nc.dram_tensor("rope", [128, NTC, 512], BF16, kind="ExternalInput").ap()
    wp_d = nc.dram_tensor("wpT", [128, 2, 1024], BF16, kind="ExternalInput").ap()
    ones1_d = nc.dram_tensor("ones1", [33, 64], BF16, kind="ExternalInput").ap()
    ones1f_d = nc.dram_tensor("ones1f", [33, 64], F32, kind="ExternalInput").ap()
    yT_d = nc.dram_tensor("yT", [1024, TL], BF16, kind="ExternalOutput").ap()

    with tile.TileContext(nc) as tc, ExitStack() as ctx:
        sb = ctx.enter_context(tc.tile_pool(name="sb", bufs=1))
        ptp = ctx.enter_context(tc.tile_pool(name="ptp", bufs=6))
        ytp = ctx.enter_context(tc.tile_pool(name="ytp", bufs=8))

        xT = sb.tile([128, NTC, NCT, 512], BF16)
        wqk = sb.tile([128, 4, NCT, 128], BF16)
        wv = sb.tile([128, NCT, 256], BF16)
        bqk = sb.tile([128, 4], F32)
        bv = sb.tile([128, 256], F32)
        rope = sb.tile([128, NTC, 512], BF16)
        mi = sb.tile([128, 4, 1024], F16)  # c - p - 128d; mask = (mi >= 0)
        wp = sb.tile([128, 2, 1024], BF16)
        q_sb = sb.tile([128, 2, TL], BF16)
        k_sb = sb.tile([128, 2, TL], BF16)
        v_sb = sb.tile([128, 4 * NTT, 65], BF16)
        attT = sb.tile([128, 2, TL], F32)
        attbf = sb.tile([128, 2, TL], BF16)
        rs2 = sb.tile([128, TL], F32)    # rows 32*h hold chunk rowsums
        rs2r = sb.tile([128, TL], F32)
        ones1 = sb.tile([33, 64], BF16)  # bcast lhsT: ones at rows 0 and 32
        ones1f = sb.tile([33, 64], F32)  # f32 twin for the final (tail) chunk
        rs2rb = sb.tile([33, TL], BF16)  # bf16 reciprocal rows for the bcast

        # critical path first: B block0 needs wqk m=2,0 + x t4=0 + rope0 +
        # bqk; C tiles 0-3 need wv/bv. Later tensors stream behind on the 4
        # DGE rings in need order. Masks are computed on the (idle) gpsimd
        # engine instead of DMA: val[p,c] = c - p - 128d, mask = (val >= 0).
        nc.sync.dma_start(out=wqk[:, 2], in_=wqk_d[:, 2])
        nc.gpsimd.dma_start(out=wqk[:, 0], in_=wqk_d[:, 0])
        nc.scalar.dma_start(out=bqk, in_=bqk_d)
        nc.scalar.dma_start(out=rope[:, 0], in_=rope_d[:, 0])
        nc.sync.dma_start(out=xT[:, 0, 0:4], in_=xT_d[:, 0, 0:4])
        nc.gpsimd.dma_start(out=xT[:, 0, 4:8], in_=xT_d[:, 0, 4:8])
        nc.sync.dma_start(out=wqk[:, 3], in_=wqk_d[:, 3])
        nc.scalar.dma_start(out=wv, in_=wv_d)
        nc.scalar.dma_start(out=bv, in_=bv_d)
        nc.vector.memset(v_sb[:, :, 64:65], 1.0)
        nc.vector.memset(rs2, 1.0)  # keep unused partitions finite for recip
        for d in range(4):
            nc.gpsimd.iota(mi[:, d, :], [[0, 2], [1, 512]], base=-128 * d,
                           channel_multiplier=-1,
                           allow_small_or_imprecise_dtypes=True)
            if d == 0:
                nc.gpsimd.dma_start(out=wqk[:, 1], in_=wqk_d[:, 1])
            elif d == 1:
                nc.sync.dma_start(out=xT[:, 1], in_=xT_d[:, 1])
            elif d == 2:
                nc.gpsimd.dma_start(out=xT[:, 3], in_=xT_d[:, 3])
        nc.sync.dma_start(out=xT[:, 2], in_=xT_d[:, 2])
        for t4 in range(1, NTC):
            nc.scalar.dma_start(out=rope[:, t4], in_=rope_d[:, t4])
        nc.scalar.dma_start(out=ones1, in_=ones1_d)
        nc.scalar.dma_start(out=ones1f, in_=ones1f_d)
        nc.scalar.dma_start(out=wp, in_=wp_d)

        def phase_b_block(ms, t4, pool):
            """QK projection for one 512-col t-chunk of the given m-tiles."""
            sl = slice(t4 * 512, (t4 + 1) * 512)
            for fi, m in enumerate(ms):
                ps = pool.tile([128, 512], F32, tag=f"f{fi}", name=f"pbb_{m}_{t4}")
                for ct in range(NCT):
                    nc.tensor.matmul(
                        ps, wqk[:, m, ct, :], xT[:, t4, ct, :],
                        start=(ct == 0), stop=(ct == NCT - 1),
                        skip_group_check=True)
                dest = q_sb if m < 2 else k_sb
                nc.vector.scalar_tensor_tensor(
                    out=dest[:, m % 2, sl], in0=ps, scalar=bqk[:, m:m + 1],
                    in1=rope[:, t4, :], op0=ALU.add, op1=ALU.mult)

        def phase_b_half(m, t4, half, state, pb, tag):
            """One half (4 of 8 contraction MMs) of the QK projection for
            (m-tile, t-chunk); the second half adds the bias*rope STT."""
            if half == 0:
                state[(m, t4)] = pb.tile(
                    [128, 512], F32, tag=tag, name=f"pbf_{m}_{t4}")
            ps = state[(m, t4)]
            for ct in range(4 * half, 4 * half + 4):
                nc.tensor.matmul(
                    ps, wqk[:, m, ct, :], xT[:, t4, ct, :],
                    start=(ct == 0), stop=(ct == NCT - 1),
                    skip_group_check=True)
            if half == 1:
                dest = q_sb if m < 2 else k_sb
                sl = slice(t4 * 512, (t4 + 1) * 512)
                nc.vector.scalar_tensor_tensor(
                    out=dest[:, m % 2, sl], in0=ps, scalar=bqk[:, m:m + 1],
                    in1=rope[:, t4, :], op0=ALU.add, op1=ALU.mult)

        def phase_c_tile(tt, pool):
            """V projection for one token tile (8 MMs + fused bias add)."""
            ps = pool.tile([128, 512], F32, tag=f"f{tt % 2}", name=f"pc_{tt}")[:, 0:256]
            t4, r = divmod(tt, 4)
            for ct in range(NCT):
                nc.tensor.matmul(
                    ps, xT[:, t4, ct, r * 128:(r + 1) * 128], wv[:, ct, :],
                    start=(ct == 0), stop=(ct == NCT - 1),
                    skip_group_check=True)
            # one fused add for all 4 units of this token tile:
            # psum cols (hp*128+h*64) map to v_sb units u=4tt+2hp+h in order
            nc.vector.tensor_add(
                v_sb[:, 4 * tt:4 * tt + 4, 0:64],
                ps.rearrange("p (a b) -> p a b", a=4),
                bv.rearrange("p (a b) -> p a b", a=4))

        def norm_chunk(hp, qc, pool, last=False):
            """attbf[:, hp, qc-chunk] = attT * 1/rowsum. The reciprocal rows
            (head h at partition 32*h -- engine APs only allow base
            partitions 0/32/64; rows are reused across the temporally
            separated head pairs) are broadcast across the two 64-partition
            head blocks by contraction-1 PE matmuls with a ones row -- no
            DRAM bounce.
            reciprocal_approx_fast runs full-partition (custom-DVE op
            mishandles partition-sliced APs); other rows are finite (memset).
            """
            qsl = slice(qc * 512, (qc + 1) * 512)
            nc.vector.reciprocal_approx_fast(rs2r[:, qsl], rs2[:, qsl])
            rsb = pool.tile([128, 512], F32, tag="f0", name=f"rsb_{hp}_{qc}")
            for h in range(2):
                r = 32 * h
                if last:  # skip the converts on the drain-critical chain
                    nc.tensor.matmul(rsb[h * 64:(h + 1) * 64, :],
                                     ones1f[r:r + 1, :], rs2r[r:r + 1, qsl],
                                     skip_group_check=True)
                    continue
                # bf16 moving operand: 1 PE cycle/row instead of f32's 4
                nc.vector.tensor_copy(rs2rb[r:r + 1, qsl], rs2r[r:r + 1, qsl])
                nc.tensor.matmul(rsb[h * 64:(h + 1) * 64, :],
                                 ones1[r:r + 1, :], rs2rb[r:r + 1, qsl],
                                 skip_group_check=True)
            nc.vector.tensor_mul(attbf[:, hp, qsl], attT[:, hp, qsl], rsb)

        def phase_d(hp, pds, pdv, fill, fillers, copies_fn,
                    pre_qc=None, qc_order=(0, 1, 2, 3)):
            """Attention for head pair hp. One kt per S^T/exp group so the
            multi-buffered psum keeps PE(S) / ACT(exp) / PE(PV) overlapped.
            Diagonal tiles d = kt-4qc only have live entries in q-columns
            >= 128d, so S/exp/mask/PV are all narrowed to [128d:512) per
            head (exp/mask via a 3D two-block AP). fillers: independent PE
            work chunks consumed greedily (one per kt, destructively so the
            caller sees leftovers). Normalization runs per qc right after
            the PV copies. att_copy: engine callable for the psum->attT
            drain (scalar in D0 where exp has slack, vector in D1)."""
            for qi, qc in enumerate(qc_order):
                if pre_qc is not None:
                    pre_qc(qi)
                qsl = slice(qc * 512, (qc + 1) * 512)
                pv = [pdv.tile([65, 512], F32, tag=f"pv{h}", name=f"pv_{qc}_{hp}_{h}")
                      for h in range(2)]
                n_kt = 4 * (qc + 1)
                for _ in range(2 if n_kt <= 4 else 1):
                    if fillers:
                        fillers.pop(0)()  # ahead of S(kt0): covers psum waits
                qstride = max(1, n_kt // len(fillers)) if fillers else 1
                for kt in range(n_kt):
                    ksl = slice(kt * 128, (kt + 1) * 128)
                    d = kt - 4 * qc if kt >= 4 * qc else None
                    off = 128 * d if d else 0
                    sps = pds.tile([128, 1024], F32, tag="sps", name=f"sps_{qc}_{hp}_{kt}")
                    for h in range(2):
                        hsl = slice(h * 64, (h + 1) * 64)
                        nc.tensor.matmul(
                            sps[:, h * 512 + off:(h + 1) * 512],
                            k_sb[hsl, hp, ksl],
                            q_sb[hsl, hp, qc * 512 + off:(qc + 1) * 512])
                    pt = ptp.tile([128, 1024], BF16, tag="pt", name=f"pt_{qc}_{hp}_{kt}")
                    if off:
                        pt_r = pt.rearrange("p (h c) -> p h c", h=2)[:, :, off:512]
                        sps_r = sps.rearrange("p (h c) -> p h c", h=2)[:, :, off:512]
                        nc.scalar.activation(pt_r, sps_r, AF.Exp, bias=0.0, scale=0.125)
                    else:
                        nc.scalar.activation(pt, sps, AF.Exp, bias=0.0, scale=0.125)
                    if d is not None:  # partial (diagonal) tile: mask both heads
                        pt_m = pt.rearrange("p (h c) -> p h c", h=2)[:, :, off:512]
                        msk = mi[:, d, :].rearrange("p (h c) -> p h c", h=2)[:, :, off:512]
                        nc.vector.scalar_tensor_tensor(
                            out=pt_m, in0=msk, scalar=0.0, in1=pt_m,
                            op0=ALU.is_ge, op1=ALU.mult)
                    # filler sits between S and PV in the PE FIFO, covering
                    # the exp(+mask) latency that PV(kt) must wait out
                    if fillers and kt % qstride == qstride - 1:
                        fillers.pop(0)()
                    for h in range(2):
                        u = (kt * 2 + hp) * 2 + h
                        nc.tensor.matmul(
                            pv[h][:, off:512], v_sb[:, u, :],
                            pt[:, h * 512 + off:(h + 1) * 512],
                            start=(kt == 0), stop=(kt == n_kt - 1),
                            skip_group_check=True)
                att_copy, rs_copy = copies_fn(qi)
                for h in range(2):
                    r = 32 * h
                    rs_copy(rs2[r:r + 1, qsl], pv[h][64:65, :])
                for h in range(2):
                    att_copy(attT[h * 64:(h + 1) * 64, hp, qsl], pv[h][0:64, :])
                norm_chunk(hp, qc, fill, last=(hp == 1 and qi == 3))

        def phase_e_pair(mp, t4, pool, copy_engines, tail=False):
            """Output projection for 2 m-tiles of one 512-col t-chunk:
            y^T = WpT.T @ attbf, psum -> bf16 -> DRAM."""
            for i in range(2):
                mt = 2 * mp + i
                ps = pool.tile([128, 512], F32, tag=f"f{(mp + i) % 2}",
                               name=f"pe_{t4}_{mt}")
                for hp in range(2):
                    nc.tensor.matmul(
                        ps, wp[:, hp, mt * 128:(mt + 1) * 128],
                        attbf[:, hp, t4 * 512:(t4 + 1) * 512],
                        start=(hp == 0), stop=(hp == 1), skip_group_check=True)
                yt = ytp.tile([128, 512], BF16, tag="yt", name=f"yt_{mt}_{t4}")
                copy_engines[i](yt, ps)
                ring = (nc.sync, nc.gpsimd)[i]
                ring.dma_start(
                    out=yT_d[mt * 128:(mt + 1) * 128, t4 * 512:(t4 + 1) * 512],
                    in_=yt)

        with tc.tile_pool(name="fill", bufs=1, space="PSUM") as fill, \
             tc.tile_pool(name="pds", bufs=2, space="PSUM") as pds, \
             tc.tile_pool(name="pdv", bufs=1, space="PSUM") as pdv:
            # head-pair-0 QK t-chunk 0 + v tiles for D0 qc0, emitted up front
            phase_b_block((2, 0), 0, fill)
            for tt in range(4):
                phase_c_tile(tt, fill)
            # remaining v tiles + head-pair-1 QK interleave into D0's gaps,
            # ordered by the x t-chunk they need so the filler stream never
            # outpaces the input DMA stream (PE FIFO head-of-line blocking)
            bstate = {}
            fillers = []
            fi = [0]

            def b_half(m, t4, half):
                tag = f"f{fi[0] % 2}"
                if half == 1:
                    fi[0] += 1
                return lambda: phase_b_half(m, t4, half, bstate, fill, tag)

            # D0 fillers: B-hp1 t-chunk 0 (must precede D1 qc0) + remaining v
            # tiles, grouped by the x t-chunk they need so the filler stream
            # never outpaces the input DMA stream.
            for m in (3, 1):
                for half in range(2):
                    fillers.append(b_half(m, 0, half))
            for tt in range(4, NTT):
                fillers.append(lambda tt=tt: phase_c_tile(tt, fill))

            def b0_pre(qc):
                if qc > 0:
                    phase_b_block((2, 0), qc, fill)

            # copies: scalar while its exp pipe has slack; the last qc's
            # copies feed the D0->D1 boundary, keep them off scalar's queue
            phase_d(0, pds, pdv, fill, fillers,
                    lambda qi: ((nc.vector.tensor_copy, nc.vector.tensor_copy)
                                if qi == 3 else (nc.scalar.copy, nc.scalar.copy)),
                    pre_qc=b0_pre)
            for f in list(fillers):
                f()
            fillers.clear()

            # D1 is exp-bound: feed it the remaining B-hp1 chunks (t-chunk k
            # only needed from qc k on) and phase E per chunk as soon as that
            # chunk's normalize has run. E psum->bf16 copies go to vector
            # while exp is hot, scalar once exp drains.
            e_fillers = []
            for m in (3, 1):
                for half in range(2):
                    e_fillers.append(b_half(m, 1, half))

            def d1_pre(qc):
                if qc in (1, 2):
                    for m in (3, 1):
                        for half in range(2):
                            e_fillers.append(b_half(m, qc + 1, half))
                if qc > 0:
                    ce = (nc.vector.tensor_copy, nc.vector.tensor_copy)
                    e_fillers.extend(
                        (lambda mp=mp, t4=qc - 1: phase_e_pair(mp, t4, fill, ce))
                        for mp in range(4))

            # last qc: attT drain on (now idle) scalar in parallel with the
            # vector-side rs->recip->bcast chain that gates the final E chunk
            phase_d(1, pds, pdv, fill, e_fillers,
                    lambda qi: ((nc.scalar.copy, nc.vector.tensor_copy)
                                if qi == 3 else
                                (nc.vector.tensor_copy, nc.vector.tensor_copy)),
                    pre_qc=d1_pre)
            for f in list(e_fillers):
                f()
            e_fillers.clear()
        # final E chunk: pds/pdv banks are free now; a 4-buffer psum pool
        # removes the 2-bank copy/matmul round-robin stall in the drain
        with tc.tile_pool(name="pe4", bufs=2, space="PSUM") as pe4:
            for mp in range(4):
                phase_e_pair(mp, 3, pe4, (nc.vector.tensor_copy, nc.scalar.copy),
                             tail=True)
    nc.compile()
    return nc


def _rope_T():
    theta = 1.0 / (10000.0 ** (2.0 * np.arange(0, HD // 2, dtype=np.float32) / HD))
    seq = np.arange(1, T + 1, dtype=np.float32)
    ang = np.einsum('n,d->nd', seq, theta)
    ang = np.concatenate([ang, ang], axis=-1)
    f = (np.cos(ang) + np.sin(ang)).astype(np.float32)  # [T, 64]
    return np.concatenate([f.T, f.T], axis=0)           # [128, T]


def _host_inputs(x, W_attn, b_attn, W_proj, b_proj):
    bf = ml_dtypes.bfloat16
    ropeT = np.ascontiguousarray(_rope_T().reshape(128, NTC, 512))
    ones1 = np.zeros((33, 64), bf)
    ones1[0, :] = 1.0
    ones1[32, :] = 1.0
    ones1f = ones1.astype(np.float32)

    in_maps = []
    for c in range(8):
        b, j = divmod(c, 4)
        hs = [4 * j + i for i in range(4)]
        xT = np.ascontiguousarray(x[b].T).astype(bf)          # [1024, TL]
        q_rows = np.concatenate([W_attn[64 * h:64 * (h + 1)] for h in hs], 0)
        k_rows = np.concatenate([W_attn[C + 64 * h:C + 64 * (h + 1)] for h in hs], 0)
        WqkT = np.concatenate([q_rows, k_rows], 0).T          # [1024, 512]
        bqk = np.concatenate(
            [np.concatenate([b_attn[64 * h:64 * (h + 1)] for h in hs]),
             np.concatenate([b_attn[C + 64 * h:C + 64 * (h + 1)] for h in hs])])
        v_rows = np.concatenate([W_attn[2 * C + 64 * h:2 * C + 64 * (h + 1)] for h in hs], 0)
        WvT = v_rows.T                                        # [1024, 256]
        bv = np.concatenate([b_attn[2 * C + 64 * h:2 * C + 64 * (h + 1)] for h in hs])
        WpT = np.concatenate([W_proj[:, 64 * h:64 * (h + 1)] for h in hs], 1).T  # [256,1024]
        in_maps.append({
            "xT": np.ascontiguousarray(
                xT.reshape(NCT, 128, NTC, 512).transpose(1, 2, 0, 3)),
            "wqkT": np.ascontiguousarray(
                WqkT.astype(bf).reshape(NCT, 128, 4, 128).transpose(1, 2, 0, 3)),
            "wvT": np.ascontiguousarray(
                WvT.astype(bf).reshape(NCT, 128, 256).transpose(1, 0, 2)),
            "bqk": np.ascontiguousarray(bqk.reshape(4, 128).T.astype(np.float32)),
            "bv": np.ascontiguousarray(
                np.broadcast_to(bv[None, :].astype(np.float32), (128, 256))),
            "rope": ropeT.astype(bf),
            "ones1": ones1,
            "ones1f": ones1f,
            "wpT": np.ascontiguousarray(
                WpT.astype(bf).reshape(2, 128, 1024).transpose(1, 0, 2)),
        })
    return in_maps


def kernel(x, W_attn, b_attn, W_proj, b_proj):
    if "nc" not in _CACHE:
        _CACHE["nc"] = _build_nc()
    nc = _CACHE["nc"]
    in_maps = _host_inputs(x, W_attn, b_attn, W_proj, b_proj)
    res = run_bass_kernel_spmd(nc, in_maps, list(range(8)), trace=TRACE)
    _CACHE["last"] = res
    y = np.zeros((B, T, C), np.float32)
    for c in range(8):
        y[c // 4] += res.results[c]["yT"].astype(np.float32).T
    y += b_proj.astype(np.float32)
    return y
